# revision 18
# baseline (speedup 1.0000x reference)
"""Dense associative-embedding loss on 8 Trainium2 NeuronCores.

Math (reference):
    g[b, n, p, c] = pred[b, c, inds[b, n, p]]
    centers       = mean_p(g)                              # [B, N, C]
    pull          = 0.25 * sum_{b,n} sum_c (mean_p g^2 - centers^2)
    s[b, n]       = sum_c centers
    push          = 0.25 * sum_b sum_{i != j} relu(2 - |s_i - s_j|) / (N(N-1))

Only B*N*P*C = 262144 of pred's 33.5M elements are ever read, so the kernel
is a sparse gather. The host re-lays pred channel-last ([b, hw, c] flat), so
each point's 8 channels are one contiguous 32-byte run. On-chip, indirect
DMAs gather 128 points per instruction (the HW contract is one descriptor
per SBUF partition, descriptor length = dest row size): 32 instructions
fill g[128, 256] with point slot (p, k) at partition p = b*64 + n*2 + pp//32,
col k = pp % 32 (2 partitions per instance).

Reduction: per-partition strided X-reduces give R1 = sum_k g, R2 = sum_k g^2
per (partition, channel); one small fp32 matmul against a 0/1 instance
indicator contracts the two partitions of each instance -> S1|S2 [64, 16].
The push pairwise term replicates s across partitions with one PE
transpose of the free-broadcast s, then masks with a -1/0 block mask. Per-instance partials [64, 2] go to the host, which applies
the affine normalization and sums across cores (the unshard step).
"""

import numpy as np

_B, _C, _H, _W = 16, 8, 512, 512
_HW = _H * _W
_N, _P = 32, 64
_NCORES = 8
_BP = _B // _NCORES              # batch elements per core
_NI = _BP * _N                   # instances per core = 64
_KCOLS = 32                      # point slots per partition
_NGATHER = _P // _KCOLS          # partitions per instance = 2
_V = _BP * _HW * _C              # flat pred elements per core (channel-last)

_MARGIN = 2.0
_PULL_W = 0.25
_PUSH_W = 0.25

_program = None


def _build_program():
    import concourse.bacc as bacc
    import concourse.bass as bass
    import concourse.mybir as mybir
    import concourse.tile as tile

    f32 = mybir.dt.float32
    i32 = mybir.dt.int32
    X = mybir.AxisListType.X
    Alu = mybir.AluOpType

    nc = bacc.Bacc("TRN2", target_bir_lowering=False, debug=False)

    pred_d = nc.dram_tensor("pred", [_V, 1], f32, kind="ExternalInput")
    idx_d = nc.dram_tensor("idx", [128, _KCOLS], i32, kind="ExternalInput")
    const_d = nc.dram_tensor("aux", [128, 128], f32, kind="ExternalInput")
    out_d = nc.dram_tensor("out", [_NI, 2], f32, kind="ExternalOutput")

    with tile.TileContext(nc) as tc:
        with (
            tc.tile_pool(name="sb", bufs=1) as sb,
            tc.tile_pool(name="rq", bufs=2) as rqp,
            tc.tile_pool(name="ps", bufs=1, space="PSUM") as ps,
        ):
            idx_t = sb.tile([128, _KCOLS], i32)
            nc.sync.dma_start(out=idx_t[:], in_=idx_d[:])
            aux_t = sb.tile([128, 128], f32)
            nc.sync.dma_start(out=aux_t[:], in_=const_d[:])
            ind = aux_t[:, 0:64]          # [128, 64] instance indicator
            ident = aux_t[0:64, 64:128]   # [64, 64] identity

            # Gather in 4 chunks of 8 point-columns; each chunk's square,
            # strided X-reduces, and PSUM-accumulating matmul run under the
            # shadow of the remaining gathers.
            NCHUNK = 4
            KC = _KCOLS // NCHUNK            # 8 point columns per chunk
            g = sb.tile([128, _KCOLS * _C], f32)
            g2 = sb.tile([128, _KCOLS * _C], f32)
            s_ps = ps.tile([_NI, 2 * _C], f32)
            for q in range(NCHUNK):
                for kk in range(KC):
                    k = q * KC + kk
                    nc.gpsimd.indirect_dma_start(
                        out=g[:, k * _C : (k + 1) * _C],
                        out_offset=None,
                        in_=pred_d[:, :],
                        in_offset=bass.IndirectOffsetOnAxis(
                            ap=idx_t[:, k : k + 1], axis=0
                        ),
                    )
                sl = slice(q * KC * _C, (q + 1) * KC * _C)
                nc.vector.tensor_mul(g2[:, sl], g[:, sl], g[:, sl])
                Rq = rqp.tile([128, 2 * _C], f32, tag="Rq")
                nc.vector.reduce_sum(
                    out=Rq[:, 0:_C],
                    in_=g[:, sl].rearrange("p (k c) -> p c k", c=_C),
                    axis=X,
                )
                nc.vector.reduce_sum(
                    out=Rq[:, _C:],
                    in_=g2[:, sl].rearrange("p (k c) -> p c k", c=_C),
                    axis=X,
                )
                nc.tensor.matmul(
                    out=s_ps[:], lhsT=ind, rhs=Rq[:],
                    start=(q == 0), stop=(q == NCHUNK - 1),
                )
            S = sb.tile([_NI, 2 * _C], f32)
            nc.vector.tensor_copy(S[:], s_ps[:])
            S1 = S[:, 0:_C]
            S2 = S[:, _C:]

            o_t = sb.tile([_NI, 2], f32)
            # o_t[:,0] = pull_inst[m] = sum_c (S2 - S1^2/P), fused
            cc = sb.tile([_NI, _C], f32)
            nc.vector.tensor_mul(cc[:], S1, S1)
            u = sb.tile([_NI, _C], f32)
            nc.vector.scalar_tensor_tensor(
                out=u[:], in0=cc[:], scalar=-1.0 / _P, in1=S2,
                op0=Alu.mult, op1=Alu.add, accum_out=o_t[:, 0:1],
            )

            # s_raw[m] = sum_c S1 (unscaled: margin is scaled by P instead,
            # the host divides push by P)
            s_t = sb.tile([_NI, 1], f32)
            nc.vector.reduce_sum(out=s_t[:], in_=S1, axis=X)

            # srep[m, j] = s_j: transpose of the free-broadcast s*1^T via the
            # PE transpose path (same pattern as tile_scatter_add).
            srep_ps = ps.tile([_NI, _NI], f32)
            nc.tensor.transpose(
                out=srep_ps[:],
                in_=s_t[:].broadcast_to((_NI, _NI)),
                identity=ident,
            )
            srep = sb.tile([_NI, _NI], f32)
            nc.vector.tensor_copy(srep[:], srep_ps[:])

            # t = min(|d| - 2, 0) = max(min(d-2,0), min(-d-2,0)) = -relu(2-|d|)
            diff = sb.tile([_NI, _NI], f32)
            nc.vector.tensor_sub(
                diff[:], s_t[:].broadcast_to((_NI, _NI)), srep[:]
            )
            diffr = sb.tile([_NI, _NI], f32)
            nc.vector.tensor_sub(
                diffr[:], srep[:], s_t[:].broadcast_to((_NI, _NI))
            )
            nc.vector.tensor_scalar(
                out=diff[:], in0=diff[:],
                scalar1=_MARGIN * _P, scalar2=0.0, op0=Alu.subtract, op1=Alu.min,
            )
            nc.vector.tensor_scalar(
                out=diffr[:], in0=diffr[:],
                scalar1=_MARGIN * _P, scalar2=0.0, op0=Alu.subtract, op1=Alu.min,
            )
            nc.vector.tensor_tensor(
                out=diff[:], in0=diff[:], in1=diffr[:], op=Alu.max
            )
            # negmask: -1 within own batch block, 0 across; mask then reduce
            negmask = sb.tile([_NI, _NI], f32)
            nc.vector.memset(negmask[:], 0.0)
            for b in range(_BP):
                nc.vector.memset(
                    negmask[b * _N : (b + 1) * _N, b * _N : (b + 1) * _N], -1.0
                )
            dm = sb.tile([_NI, _NI], f32)
            nc.vector.scalar_tensor_tensor(
                out=dm[:], in0=diff[:], scalar=1.0, in1=negmask[:],
                op0=Alu.mult, op1=Alu.mult, accum_out=o_t[:, 1:2],
            )
            nc.sync.dma_start(out=out_d[:], in_=o_t[:])

    nc.finalize()
    return nc


def _get_program():
    global _program
    if _program is None:
        _program = _build_program()
    return _program


def _aux_array():
    aux = np.zeros((128, 128), np.float32)
    p = np.arange(128)
    m = (p // 64) * _N + (p % 64) // _NGATHER
    aux[p, m] = 1.0
    aux[0:64, 64:128] = np.eye(64, dtype=np.float32)
    return aux


def _make_in_maps(pred, inds):
    pred = np.asarray(pred)
    inds = np.asarray(inds).astype(np.int64)
    aux = _aux_array()
    in_maps = []
    for mcore in range(_NCORES):
        psh = pred[_BP * mcore : _BP * (mcore + 1)]   # [BP, C, H, W]
        ish = inds[_BP * mcore : _BP * (mcore + 1)]   # [BP, N, P]
        # channel-last flat layout: element (b, hw, c) at ((b*HW + hw)*C + c)
        pcl = np.ascontiguousarray(
            psh.reshape(_BP, _C, _HW).transpose(0, 2, 1), dtype=np.float32
        ).reshape(_V, 1)
        # idx[p, k]: partition p = b*64 + n*2 + pp//32, col k = pp % 32
        # element offset of point (b, n, pp) = (b*HW + inds[b,n,pp]) * C
        off = (ish + (np.arange(_BP, dtype=np.int64) * _HW)[:, None, None]) * _C
        off = off.reshape(_BP, _N, _NGATHER, _KCOLS)       # pp = half*32 + k
        idx = off.transpose(0, 1, 2, 3).reshape(_BP * _N * _NGATHER, _KCOLS)
        in_maps.append(
            {
                "pred": pcl,
                "idx": np.ascontiguousarray(idx, dtype=np.int32),
                "aux": aux,
            }
        )
    return in_maps


def _combine(core_outs):
    outs = np.stack([np.asarray(o, dtype=np.float64) for o in core_outs])  # [8, 64, 2]
    pull = _PULL_W * outs[:, :, 0].sum() / _P
    push_sum = outs[:, :, 1].sum() / _P - _B * _N * _MARGIN  # drop diagonal terms
    push = _PUSH_W * push_sum / (_N * (_N - 1))
    return np.array([pull, push], dtype=np.float32)


def _run(pred, inds, **spmd_kwargs):
    """Returns (full_output, BassKernelResults)."""
    from concourse.bass_utils import run_bass_kernel_spmd

    nc = _get_program()
    in_maps = _make_in_maps(pred, inds)
    res = run_bass_kernel_spmd(nc, in_maps, core_ids=list(range(_NCORES)), **spmd_kwargs)
    return _combine([r["out"] for r in res.results]), res


def kernel(pred, inds):
    out, _ = _run(pred, inds)
    return out
